# revision 3
# baseline (speedup 1.0000x reference)
"""GaussianBlur2d Trainium2 kernel: 13x13 separable gaussian blur, reflect pad.

Input : x [32, 1, 1024, 1024] f32, kernel [1, 1, 13, 13] f32 (rank-1 separable).
Output: [32, 1, 1024, 1024] f32.

Strategy (pure data parallel, 4 images per core on 8 cores), all-bf16:
  The 2D conv is factored (SVD rank-1) into a vertical and a horizontal
  13-tap pass, both on the TensorEngine in bf16 (fp32 matmuls cost 4
  cycles/row on TRN2; bf16 costs 1, and the 2e-2 tolerance leaves bf16
  ~10x margin). The host converts x to bf16 (halves input DMA) and the
  kernel emits y^T in bf16 (halves output DMA; host untransposes).

  Pass 1 (vertical taps) keeps an IMAGE TILE stationary:
     t1[c, o] = sum_r X[r, c-window] * Bv[r, o]
  so the output arrives pre-transposed (partition=col) - exactly the
  contraction layout pass 2 needs. Windows are 128 rows at stride 116
  (6-row halo), so every output block is one matmul, no PSUM spills.
  Reflect-pad taps fold into the edge windows' band matrices.

  Pass 1 PSUM is a single [128, 1024] tile (2 banks); the one block
  crossing the col-512 bank boundary is split into two matmuls so no
  matmul write crosses a bank. That makes the PSUM->SBUF drain ONE
  copy per column group (DVE cost = free-size + fixed overhead, so
  fewer/larger copies win).

  Pass 2 (horizontal taps) keeps the BAND stationary:
     yT[o, r] = sum_c Bh[c, o] * t1[c, r]
  two N=512 matmuls per block stream the whole 1024-row extent, and
  one scalar-engine copy drains each [width, 1024] result to SBUF.
"""
import numpy as np
import ml_dtypes

import concourse.bacc as bacc
import concourse.mybir as mybir
import concourse.tile as tile
from concourse import bass_utils

F32 = mybir.dt.float32
BF16 = mybir.dt.bfloat16

H = 1024          # image rows/cols
SEG = 128         # stationary window height (contraction K)
KS = 13
HALF = KS // 2
N_CORES = 8
IMGS_PER_CORE = 4
BANK = 512        # PSUM bank width in f32 cols

# output blocks: [0,122) from the aligned first window, then stride 116,
# last block [934,1024) from the aligned last window
BLOCK_STARTS = [0] + [122 + 116 * i for i in range(7)] + [934]
BLOCK_ENDS = [122] + [122 + 116 * (i + 1) for i in range(7)] + [1024]
NBLK = 9
# stationary window first row per block (clipped to the image)
WIN_STARTS = [0] + [122 + 116 * i - HALF for i in range(7)] + [H - SEG]


def _reflect(r):
    if r < 0:
        return -r
    if r > H - 1:
        return 2 * (H - 1) - r
    return r


def _decompose_kernel(k2d):
    k = np.asarray(k2d, dtype=np.float64).reshape(KS, KS)
    u, s, vh = np.linalg.svd(k)
    gv = u[:, 0] * np.sqrt(s[0])
    gh = vh[0, :] * np.sqrt(s[0])
    if gv.sum() < 0:
        gv, gh = -gv, -gh
    return gv, gh


def _build_bands(g):
    """Band matrix [128, 1024]: col o = taps of output o within its window."""
    out = np.zeros((SEG, H), dtype=np.float64)
    for blk in range(NBLK):
        o0, o1 = BLOCK_STARTS[blk], BLOCK_ENDS[blk]
        r0 = WIN_STARTS[blk]
        for o in range(o0, o1):
            for t in range(KS):
                rr = _reflect(o - HALF + t)
                if r0 <= rr < r0 + SEG:
                    out[rr - r0, o] += g[t]
    return out.astype(ml_dtypes.bfloat16)


def _pass1_segments():
    """(blk, s0, s1) matmul segments, no segment crossing the bank edge."""
    segs = []
    for blk in range(NBLK):
        o0, o1 = BLOCK_STARTS[blk], BLOCK_ENDS[blk]
        if o0 < BANK < o1:
            segs.append((blk, o0, BANK))
            segs.append((blk, BANK, o1))
        else:
            segs.append((blk, o0, o1))
    return segs


_SEGS = _pass1_segments()
N_WARM = 10  # HAM warmup matmuls issued under the initial input DMA


def _build_program(shared_bands):
    # shared_bands: separable factors equal (symmetric kernel) -> one band
    # array serves both passes
    nbc = H if shared_bands else 2 * H
    p2off = 0 if shared_bands else H
    nc = bacc.Bacc("TRN2", target_bir_lowering=False, debug=False)
    x = nc.dram_tensor("x", [IMGS_PER_CORE, H, H], BF16, kind="ExternalInput")
    bands = nc.dram_tensor("bands", [SEG, nbc], BF16, kind="ExternalInput")
    y = nc.dram_tensor("y", [IMGS_PER_CORE, H, H], BF16, kind="ExternalOutput")

    with tile.TileContext(nc) as tc:
        with (
            tc.tile_pool(name="xp", bufs=2) as xp,
            tc.tile_pool(name="t1p", bufs=3) as t1p,
            tc.tile_pool(name="op", bufs=3) as op,
            tc.tile_pool(name="bp", bufs=1) as bp,
            tc.tile_pool(name="ps", bufs=2, space="PSUM") as psp,
        ):
            bt = bp.tile([SEG, nbc], BF16, tag="bands")
            nc.sync.dma_start(bt[:], bands[:])

            # HAM warmup: junk matmuls on the band tile keep the PE busy
            # during the first image's input DMA so real matmuls run at
            # the warm 2.4 GHz clock.
            wps = psp.tile([SEG, H], F32, name="warm", tag="ph")
            for i in range(N_WARM):
                half = (i % 2) * BANK
                nc.tensor.matmul(
                    wps[:, half:half + BANK], bt[:, 0:SEG], bt[:, 0:BANK],
                    start=(i < 2), stop=(i >= N_WARM - 2),
                )

            for b in range(IMGS_PER_CORE):
                # overlapping 128-row stationary windows (stride 116)
                xts = []
                for blk in range(NBLK):
                    r0 = WIN_STARTS[blk]
                    xs = xp.tile([SEG, H], BF16, name=f"xt{blk}", tag=f"x{blk}")
                    nc.sync.dma_start(xs[:], x[b, r0:r0 + SEG, :])
                    xts.append(xs)
                # pass 1: vertical taps; col-group cg covers image cols
                # [WIN_STARTS[cg], +128); output t1 = T1^T group [col, row]
                for cg in range(NBLK):
                    c0 = WIN_STARTS[cg]
                    ps = psp.tile([SEG, H], F32, name=f"psv{cg}", tag="pv")
                    b0 = b1 = True  # per-bank start flag
                    for (blk, s0, s1) in _SEGS:
                        st = b0 if s0 < BANK else b1
                        if s0 < BANK:
                            b0 = False
                        else:
                            b1 = False
                        nc.tensor.matmul(
                            ps[:, s0:s1],
                            xts[blk][:, c0:c0 + SEG],
                            bt[:, s0:s1],
                            start=st,
                            stop=(s1 == BANK or s1 == H),
                        )
                    t1 = t1p.tile([SEG, H], BF16, name=f"t1{cg}", tag="t1")
                    nc.vector.tensor_copy(t1[:], ps[:])

                    # pass 2 for output col block cg: band stationary,
                    # t1 moving; output y^T block [width, 1024]
                    o0 = BLOCK_STARTS[cg]
                    w = BLOCK_ENDS[cg] - o0
                    ph = psp.tile([SEG, H], F32, name=f"psh{cg}", tag="ph")
                    for half in (0, BANK):
                        nc.tensor.matmul(
                            ph[:w, half:half + BANK],
                            bt[:, p2off + o0:p2off + o0 + w],
                            t1[:, half:half + BANK],
                            start=True, stop=True,
                        )
                    yt = op.tile([SEG, H], BF16, name=f"yt{cg}", tag="yt")
                    nc.scalar.copy(yt[:w, :], ph[:w, :])
                    # output DMAs go on the Activation HWDGE ring; input
                    # DMAs stay on the SP ring so the two directions don't
                    # serialize on one descriptor-generation engine
                    nc.scalar.dma_start(y[b, o0:o0 + w, :], yt[:w, :])
    nc.compile()
    return nc


_NC_CACHE = {}


def _get_program(shared_bands):
    if shared_bands not in _NC_CACHE:
        _NC_CACHE[shared_bands] = _build_program(shared_bands)
    return _NC_CACHE[shared_bands]


def run(x, kernel, trace=False, tmpdir=None):
    """Full-input entry. Returns (y, BassKernelResults)."""
    x = np.asarray(x, dtype=np.float32).reshape(32, H, H)
    xb = np.ascontiguousarray(x).astype(ml_dtypes.bfloat16)
    gv, gh = _decompose_kernel(kernel)
    shared = bool(np.allclose(gv, gh, rtol=0, atol=1e-12 * np.abs(gv).max()))
    if shared:
        bands = _build_bands(gv)
    else:
        bands = np.concatenate([_build_bands(gv), _build_bands(gh)], axis=1)
    nc = _get_program(shared)
    in_maps = [
        {"x": xb[c * IMGS_PER_CORE:(c + 1) * IMGS_PER_CORE], "bands": bands}
        for c in range(N_CORES)
    ]
    res = bass_utils.run_bass_kernel_spmd(
        nc, in_maps, core_ids=list(range(N_CORES)), trace=trace, tmpdir=tmpdir)
    yt = np.concatenate([np.asarray(res.results[c]["y"]) for c in range(N_CORES)],
                        axis=0)
    y = np.ascontiguousarray(yt.transpose(0, 2, 1)).astype(np.float32)
    return y.reshape(32, 1, H, H), res


def kernel(x, kernel):
    y, _ = run(x, kernel, trace=False)
    return y


# revision 5
# speedup vs baseline: 1.5746x; 1.5746x over previous
"""GaussianBlur2d Trainium2 kernel: 13x13 separable gaussian blur, reflect pad.

Input : x [32, 1, 1024, 1024] f32, kernel [1, 1, 13, 13] f32 (rank-1 separable).
Output: [32, 1, 1024, 1024] f32.

Strategy (pure data parallel, 4 images per core on 8 cores), all-bf16:
  The 2D conv is factored (SVD rank-1) into a vertical and a horizontal
  13-tap pass, both on the TensorEngine in bf16 (fp32 matmuls cost 4
  cycles/row on TRN2; bf16 costs 1, and the 2e-2 tolerance leaves bf16
  ~10x margin). The host converts x to bf16 (halves input DMA) and the
  kernel emits y^T in bf16 (halves output DMA; host untransposes).

  Pass 1 (vertical taps) keeps an IMAGE TILE stationary:
     t1[c, o] = sum_r X[r, c-window] * Bv[r, o]
  so the output arrives pre-transposed (partition=col) - exactly the
  contraction layout pass 2 needs. Windows are 128 rows at stride 116
  (6-row halo), so every output block is one matmul, no PSUM spills.
  Reflect-pad taps fold into the edge windows' band matrices.

  Pass 1 PSUM is a single [128, 1024] tile (2 banks); the one block
  crossing the col-512 bank boundary is split into two matmuls so no
  matmul write crosses a bank. That makes the PSUM->SBUF drain ONE
  copy per column group (DVE cost = free-size + fixed overhead, so
  fewer/larger copies win).

  Pass 2 (horizontal taps) keeps the BAND stationary:
     yT[o, r] = sum_c Bh[c, o] * t1[c, r]
  two N=512 matmuls per block stream the whole 1024-row extent, and
  one scalar-engine copy drains each [width, 1024] result to SBUF.
"""
import numpy as np
import ml_dtypes

import concourse.bacc as bacc
import concourse.mybir as mybir
import concourse.tile as tile
from concourse import bass_utils

F32 = mybir.dt.float32
BF16 = mybir.dt.bfloat16

H = 1024          # image rows/cols
SEG = 128         # stationary window height (contraction K)
KS = 13
HALF = KS // 2
N_CORES = 8
IMGS_PER_CORE = 4
BANK = 512        # PSUM bank width in f32 cols

# Output block widths are chosen for the HWDGE descriptor-split rule:
# a DMA is split across (largest divisor of partition count <= 16) SDMA
# engines, so 112-wide (16-way) and 120-wide (15-way) blocks spread output
# descriptors across all engines; 122/116/90 widths clump onto 2-4 engines.
# Interior windows: 112 outputs + 2*6 halo + 4 slack <= 128 rows. Edge
# windows are image-aligned (reflect taps fold into their bands).
BLOCK_STARTS = [0] + [120 + 112 * i for i in range(7)] + [904]
BLOCK_ENDS = [120] + [120 + 112 * (i + 1) for i in range(7)] + [1024]
NBLK = 9
# stationary window first row per block (clipped to the image)
WIN_STARTS = [0] + [120 + 112 * i - HALF for i in range(7)] + [H - SEG]


def _reflect(r):
    if r < 0:
        return -r
    if r > H - 1:
        return 2 * (H - 1) - r
    return r


def _decompose_kernel(k2d):
    k = np.asarray(k2d, dtype=np.float64).reshape(KS, KS)
    u, s, vh = np.linalg.svd(k)
    gv = u[:, 0] * np.sqrt(s[0])
    gh = vh[0, :] * np.sqrt(s[0])
    if gv.sum() < 0:
        gv, gh = -gv, -gh
    return gv, gh


def _build_bands(g):
    """Band matrix [128, 1024]: col o = taps of output o within its window."""
    out = np.zeros((SEG, H), dtype=np.float64)
    for blk in range(NBLK):
        o0, o1 = BLOCK_STARTS[blk], BLOCK_ENDS[blk]
        r0 = WIN_STARTS[blk]
        for o in range(o0, o1):
            for t in range(KS):
                rr = _reflect(o - HALF + t)
                if r0 <= rr < r0 + SEG:
                    out[rr - r0, o] += g[t]
    return out.astype(ml_dtypes.bfloat16)


def _pass1_segments():
    """(blk, s0, s1) matmul segments, no segment crossing the bank edge."""
    segs = []
    for blk in range(NBLK):
        o0, o1 = BLOCK_STARTS[blk], BLOCK_ENDS[blk]
        if o0 < BANK < o1:
            segs.append((blk, o0, BANK))
            segs.append((blk, BANK, o1))
        else:
            segs.append((blk, o0, o1))
    return segs


_SEGS = _pass1_segments()
N_WARM = 10  # HAM warmup matmuls issued under the initial input DMA


def _build_program(shared_bands):
    # shared_bands: separable factors equal (symmetric kernel) -> one band
    # array serves both passes
    nbc = H if shared_bands else 2 * H
    p2off = 0 if shared_bands else H
    nc = bacc.Bacc("TRN2", target_bir_lowering=False, debug=False)
    x = nc.dram_tensor("x", [IMGS_PER_CORE, H, H], BF16, kind="ExternalInput")
    bands = nc.dram_tensor("bands", [SEG, nbc], BF16, kind="ExternalInput")
    y = nc.dram_tensor("y", [IMGS_PER_CORE, H, H], BF16, kind="ExternalOutput")

    with tile.TileContext(nc) as tc:
        with (
            tc.tile_pool(name="xp", bufs=2) as xp,
            tc.tile_pool(name="t1p", bufs=3) as t1p,
            tc.tile_pool(name="op", bufs=3) as op,
            tc.tile_pool(name="bp", bufs=1) as bp,
            tc.tile_pool(name="ps", bufs=2, space="PSUM") as psp,
        ):
            bt = bp.tile([SEG, nbc], BF16, tag="bands")
            nc.sync.dma_start(bt[:], bands[:])

            # HAM warmup: junk matmuls on the band tile keep the PE busy
            # during the first image's input DMA so real matmuls run at
            # the warm 2.4 GHz clock.
            wps = psp.tile([SEG, H], F32, name="warm", tag="ph")
            for i in range(N_WARM):
                half = (i % 2) * BANK
                nc.tensor.matmul(
                    wps[:, half:half + BANK], bt[:, 0:SEG], bt[:, 0:BANK],
                    start=(i < 2), stop=(i >= N_WARM - 2),
                )

            for b in range(IMGS_PER_CORE):
                # overlapping 128-row stationary windows (stride 116)
                xts = []
                for blk in range(NBLK):
                    r0 = WIN_STARTS[blk]
                    xs = xp.tile([SEG, H], BF16, name=f"xt{blk}", tag=f"x{blk}")
                    nc.sync.dma_start(xs[:], x[b, r0:r0 + SEG, :])
                    xts.append(xs)
                # pass 1: vertical taps; col-group cg covers image cols
                # [WIN_STARTS[cg], +128); output t1 = T1^T group [col, row]
                for cg in range(NBLK):
                    c0 = WIN_STARTS[cg]
                    ps = psp.tile([SEG, H], F32, name=f"psv{cg}", tag="pv")
                    b0 = b1 = True  # per-bank start flag
                    for (blk, s0, s1) in _SEGS:
                        st = b0 if s0 < BANK else b1
                        if s0 < BANK:
                            b0 = False
                        else:
                            b1 = False
                        nc.tensor.matmul(
                            ps[:, s0:s1],
                            xts[blk][:, c0:c0 + SEG],
                            bt[:, s0:s1],
                            start=st,
                            stop=(s1 == BANK or s1 == H),
                        )
                    t1 = t1p.tile([SEG, H], BF16, name=f"t1{cg}", tag="t1")
                    nc.vector.tensor_copy(t1[:], ps[:])

                    # pass 2 for output col block cg: band stationary,
                    # t1 moving; output y^T block [width, 1024]
                    o0 = BLOCK_STARTS[cg]
                    w = BLOCK_ENDS[cg] - o0
                    ph = psp.tile([SEG, H], F32, name=f"psh{cg}", tag="ph")
                    for half in (0, BANK):
                        nc.tensor.matmul(
                            ph[:w, half:half + BANK],
                            bt[:, p2off + o0:p2off + o0 + w],
                            t1[:, half:half + BANK],
                            start=True, stop=True,
                        )
                    yt = op.tile([SEG, H], BF16, name=f"yt{cg}", tag="yt")
                    nc.scalar.copy(yt[:w, :], ph[:w, :])
                    nc.sync.dma_start(y[b, o0:o0 + w, :], yt[:w, :])
    nc.compile()
    return nc


_NC_CACHE = {}


def _get_program(shared_bands):
    if shared_bands not in _NC_CACHE:
        _NC_CACHE[shared_bands] = _build_program(shared_bands)
    return _NC_CACHE[shared_bands]


def run(x, kernel, trace=False, tmpdir=None):
    """Full-input entry. Returns (y, BassKernelResults)."""
    x = np.asarray(x, dtype=np.float32).reshape(32, H, H)
    xb = np.ascontiguousarray(x).astype(ml_dtypes.bfloat16)
    gv, gh = _decompose_kernel(kernel)
    shared = bool(np.allclose(gv, gh, rtol=0, atol=1e-12 * np.abs(gv).max()))
    if shared:
        bands = _build_bands(gv)
    else:
        bands = np.concatenate([_build_bands(gv), _build_bands(gh)], axis=1)
    nc = _get_program(shared)
    in_maps = [
        {"x": xb[c * IMGS_PER_CORE:(c + 1) * IMGS_PER_CORE], "bands": bands}
        for c in range(N_CORES)
    ]
    res = bass_utils.run_bass_kernel_spmd(
        nc, in_maps, core_ids=list(range(N_CORES)), trace=trace, tmpdir=tmpdir)
    yt = np.concatenate([np.asarray(res.results[c]["y"]) for c in range(N_CORES)],
                        axis=0)
    y = np.ascontiguousarray(yt.transpose(0, 2, 1)).astype(np.float32)
    return y.reshape(32, 1, H, H), res


def kernel(x, kernel):
    y, _ = run(x, kernel, trace=False)
    return y
